# revision 30
# baseline (speedup 1.0000x reference)
"""Trainium2 Bass kernel for 3-layer GATv2 + sum-pool + MLP (nn_GAT_56977036148745).

Strategy (8 NeuronCores, SPMD), v2 — engine-balanced rewrite:
  - Nodes sharded into 8 contiguous slabs of 2048 (dst-sharding). Each core owns
    all edges whose destination lands in its slab (edges sorted by dst on host).
  - Per layer: each core computes its slab of the gather table T = h @ Wl and
    the local query table xr = h @ Wr; an AllGather assembles the full [N,256]
    bf16 table in DRAM.  Edge phase per 128-dst tile:
      pass A: per 512-edge group, PE broadcasts xr to edges via a host-built
        fp8 dst-onehot (moving operand), accumulates the channel-major gathered
        xl (identity matmul), ACT applies LeakyReLU, and per-chunk column-form
        matmuls against zero-padded att columns produce per-edge logits in one
        PSUM bank [128, 2*NC].
      between passes: one ACT exp gives per-edge softmax numerators (columns),
        one PE transpose + DVE copy gives them in row form.
      pass B: per chunk, a selector matmul expands ev rows to [128,256] PSUM,
        one group-wide DVE multiply scales the edge-major gathered xl, and one
        256-wide seg matmul per chunk against the host-built edge-onehot
        stationary accumulates numerators; a 2-wide matmul with the ev columns
        accumulates denominators in the same PSUM tile.
  - Sum-pool via graph-onehot matmuls, AllReduce of the pooled [G+1,772]
    bounce, MLP redundantly on every core (unchanged from v1).
Host preprocessing (sorting, padding, onehot construction, weight folding) is
not part of the measured device time.
"""

import sys

for _p in ("/opt/trn_rl_repo", "/root/.axon_site/_ro/trn_rl_repo"):
    if _p not in sys.path:
        sys.path.append(_p)

import numpy as np
import ml_dtypes

try:  # NTFF profiling hook shim (image's antenv lacks axon_hooks)
    import antenv.axon_hooks  # noqa: F401
except ImportError:
    import types as _types

    try:
        import trn_agent_boot.trn_boot as _tb
        _ntff_hook = _tb._ntff_profile_via_ctypes("/opt/axon/libaxon_pjrt.so")
    except Exception:
        _ntff_hook = None
    _m = _types.ModuleType("antenv.axon_hooks")
    _m.get_axon_ntff_profile_hook = lambda: _ntff_hook
    _m.set_axon_ntff_profile_hook = lambda h: None
    sys.modules["antenv.axon_hooks"] = _m

import concourse.bacc as bacc
import concourse.bass as bass
import concourse.mybir as mybir
import concourse.tile as tile
from concourse.bass import IndirectOffsetOnAxis
from concourse.bass_utils import run_bass_kernel_spmd

BF = ml_dtypes.bfloat16
F8 = ml_dtypes.float8_e4m3fn
F32 = mybir.dt.float32
BF16 = mybir.dt.bfloat16
FP8 = mybir.dt.float8e4
I16 = mybir.dt.int16
I32 = mybir.dt.int32

P = 128          # partitions / dst-tile size / edge-chunk size
H = 2            # heads
C = 128          # channels per head
D = H * C        # 256
AF = mybir.ActivationFunctionType
ALU = mybir.AluOpType
NEG_SLOPE = 0.2


class Cfg:
    def __init__(self, n, g, ndev, nchunk, in_ch=128):
        self.n = n                  # total nodes
        self.g = g                  # graphs
        self.ndev = ndev
        self.nchunk = nchunk        # edge chunks (of 128) per dst tile, mult of 4
        self.in_ch = in_ch          # layer-1 input channels
        self.npd = n // ndev        # nodes per device
        self.nt = self.npd // P     # dst/node tiles per device
        self.pda = 772              # padded pool dim (768 + cnt + pad)


# ----------------------------------------------------------------------------
# device program
# ----------------------------------------------------------------------------

def build_program(cfg: Cfg, fb2: float):
    nc = bacc.Bacc("TRN2", target_bir_lowering=False, debug=False,
                   num_devices=cfg.ndev, num_swdge_queues=4)
    NT, NC, G, NPD = cfg.nt, cfg.nchunk, cfg.g, cfg.npd
    NE = NC * P                      # padded edges per dst tile
    NG = NC // 4                     # 512-edge groups per tile
    KC1 = cfg.in_ch // P             # layer-1 K chunks (1)
    PDA = cfg.pda

    dt = nc.dram_tensor
    xT_d = dt("xT", [cfg.in_ch, NPD], BF16, kind="ExternalInput")
    wl_d = [dt(f"wl{l}", [cfg.in_ch if l == 0 else D, D], BF16, kind="ExternalInput") for l in range(3)]
    wr_d = [dt(f"wr{l}", [cfg.in_ch if l == 0 else D, D], BF16, kind="ExternalInput") for l in range(3)]
    rl_d = [None] + [dt(f"rl{l}", [1, D], BF16, kind="ExternalInput") for l in (1, 2)]
    rr_d = [None] + [dt(f"rr{l}", [1, D], BF16, kind="ExternalInput") for l in (1, 2)]
    attr_d = dt("attr", [P, 12], BF16, kind="ExternalInput")
    sel_d = dt("sel", [NC * 2, NC // 2, 2 * D], BF16, kind="ExternalInput")
    idxw_d = dt("idxw", [NT, P, NC // 4, 32], I16, kind="ExternalInput")
    oh_d = dt("oh", [NT, P, NC, P], FP8, kind="ExternalInput")
    ohd_d = dt("ohd", [NT, P, NE], FP8, kind="ExternalInput")
    ident8_d = dt("ident8", [P, P], FP8, kind="ExternalInput")
    bloc_d = dt("bloc", [P, NT], F32, kind="ExternalInput")
    poolidx_d = dt("poolidx", [P, 1], I32, kind="ExternalInput")
    fW1_d = dt("fW1p", [PDA, 768], F32, kind="ExternalInput")
    fb1_d = dt("fb1p", [P, 6], F32, kind="ExternalInput")
    fW2_d = dt("fW2p", [P, 6], F32, kind="ExternalInput")
    identbf_d = dt("identbf", [P, P], BF16, kind="ExternalInput")
    identf_d = dt("identf", [P, P], F32, kind="ExternalInput")
    iotarow_d = dt("iotarow", [P, P], BF16, kind="ExternalInput")
    ones1p_d = dt("ones1p", [1, P], BF16, kind="ExternalInput")
    onescol_d = dt("onescol", [P, 1], BF16, kind="ExternalInput")
    y_d = dt("y", [1, G], F32, kind="ExternalOutput")

    rg = [list(range(cfg.ndev))]

    with tile.TileContext(nc) as tc:
        with (
            tc.tile_pool(name="persist", bufs=1) as pp,
            tc.tile_pool(name="dram", bufs=2, space="DRAM") as dram,
            tc.tile_pool(name="oh", bufs=2) as ohpool,
            tc.tile_pool(name="ohd", bufs=2) as ohdpool,
            tc.tile_pool(name="gath", bufs=2) as gpool,
            tc.tile_pool(name="gt", bufs=3) as gtpool,
            tc.tile_pool(name="tsb", bufs=1) as tpool,
            tc.tile_pool(name="sxl", bufs=2) as sxlpool,
            tc.tile_pool(name="work", bufs=2) as wpool,
            tc.tile_pool(name="pm", bufs=1, space="PSUM") as pmpool,
            tc.tile_pool(name="plog", bufs=1, space="PSUM") as plogpool,
            tc.tile_pool(name="pevx", bufs=1, space="PSUM") as pevxpool,
            tc.tile_pool(name="pseg", bufs=1, space="PSUM") as psegpool,
            tc.tile_pool(name="pmisc", bufs=1, space="PSUM") as pmiscpool,
        ):
            # ---- persistent SBUF ----
            identbf = pp.tile([P, P], BF16, tag="identbf")
            identf = pp.tile([P, P], F32, tag="identf")
            iotarow = pp.tile([P, P], BF16, tag="iotarow")
            ones1p = pp.tile([1, P], BF16, tag="ones1p")
            onescol = pp.tile([P, 1], BF16, tag="onescol")
            attr_sb = pp.tile([P, 12], BF16, tag="attr")
            sel_sb = pp.tile([NC * 2, NC // 2, 2 * D], BF16, tag="sel")
            ident8 = pp.tile([P, P], FP8, tag="ident8")
            xT_sb = pp.tile([cfg.in_ch, NPD], BF16, tag="xT")
            hT_sb = pp.tile([P, 2, NPD], BF16, tag="hT")
            xr_sb = pp.tile([P, NT, D], BF16, tag="xr")
            pool_sb = pp.tile([P, PDA], F32, tag="pool")
            bloc_sb = pp.tile([P, NT], F32, tag="bloc")
            poolidx_sb = pp.tile([P, 1], I32, tag="poolidx")
            wl_sb = [pp.tile([P, (cfg.in_ch if l == 0 else D) // P, D], BF16, name=f"wl{l}", tag=f"wl{l}") for l in range(3)]
            wr_sb = [pp.tile([P, (cfg.in_ch if l == 0 else D) // P, D], BF16, name=f"wr{l}", tag=f"wr{l}") for l in range(3)]
            rl_sb = [None, pp.tile([1, D], BF16, name="rl1", tag="rl1"), pp.tile([1, D], BF16, name="rl2", tag="rl2")]
            rr_sb = [None, pp.tile([1, D], BF16, name="rr1", tag="rr1"), pp.tile([1, D], BF16, name="rr2", tag="rr2")]
            zero_sb = pp.tile([P, PDA], F32, tag="zero")

            for sb, d in ((identbf, identbf_d), (identf, identf_d),
                          (iotarow, iotarow_d), (ones1p, ones1p_d),
                          (onescol, onescol_d), (attr_sb, attr_d),
                          (sel_sb, sel_d), (xT_sb, xT_d), (bloc_sb, bloc_d),
                          (poolidx_sb, poolidx_d), (ident8, ident8_d)):
                nc.sync.dma_start(sb[:], d[:])
            for l in range(3):
                nc.sync.dma_start(wl_sb[l][:], wl_d[l].ap().rearrange("(k p) d -> p k d", p=P))
                nc.sync.dma_start(wr_sb[l][:], wr_d[l].ap().rearrange("(k p) d -> p k d", p=P))
                if l > 0:
                    nc.sync.dma_start(rl_sb[l][:], rl_d[l][:])
                    nc.sync.dma_start(rr_sb[l][:], rr_d[l][:])
            nc.vector.memset(pool_sb[:], 0.0)
            nc.vector.memset(zero_sb[:], 0.0)

            # pool bounce (zeroed before scatter)
            poolb_in = dram.tile([G + 1, PDA], F32, tag="poolb_in")
            poolb_out = dram.tile([G + 1, PDA], F32, tag="poolb_out")
            for r0 in range(0, G + 1, P):
                rows = min(P, G + 1 - r0)
                nc.sync.dma_start(poolb_in[r0:r0 + rows, :], zero_sb[:rows, :])

            # ------------------------------------------------------------------
            for l in range(3):
                kcs = KC1 if l == 0 else 2
                src_sb = xT_sb if l == 0 else hT_sb

                def src_lhsT(kc, nt):
                    if l == 0:
                        return src_sb[:, nt * P:(nt + 1) * P]
                    return src_sb[:, kc, nt * P:(nt + 1) * P]

                # ---- table slab + XR slab ----
                slab = dram.tile([NPD, D], BF16, tag="slab")
                Tfull = dram.tile([cfg.n, D], BF16, tag="Tfull")
                for nt in range(NT):
                    ptab = pmiscpool.tile([P, D], F32, tag="ptab")
                    for kc in range(kcs):
                        nc.tensor.matmul(ptab[:], src_lhsT(kc, nt), wl_sb[l][:, kc, :],
                                         start=(kc == 0), stop=(kc == kcs - 1 and l == 0))
                    if l > 0:
                        nc.tensor.matmul(ptab[:], ones1p[:], rl_sb[l][:], start=False, stop=True)
                    tab = wpool.tile([P, D], BF16, tag="tab")
                    nc.vector.tensor_copy(tab[:], ptab[:])
                    nc.sync.dma_start(slab[nt * P:(nt + 1) * P, :], tab[:])

                    pxr = pmiscpool.tile([P, D], F32, tag="ptab")
                    for kc in range(kcs):
                        nc.tensor.matmul(pxr[:], src_lhsT(kc, nt), wr_sb[l][:, kc, :],
                                         start=(kc == 0), stop=(kc == kcs - 1 and l == 0))
                    if l > 0:
                        nc.tensor.matmul(pxr[:], ones1p[:], rr_sb[l][:], start=False, stop=True)
                    nc.vector.tensor_copy(xr_sb[:, nt, :], pxr[:])

                nc.gpsimd.collective_compute(
                    "AllGather", ALU.bypass, replica_groups=rg,
                    ins=[slab.opt()], outs=[Tfull.opt()],
                )

                # ---- edge phase ----
                ppool = pmiscpool.tile([P, D + 1], F32, tag="ppool")
                for t in range(NT):
                    idx_sb = wpool.tile([P, NC // 4, 32], I16, tag="idx")
                    nc.sync.dma_start(idx_sb[:], idxw_d[t])
                    oh_sb = ohpool.tile([P, NC, P], FP8, tag="oh")
                    nc.sync.dma_start(oh_sb[:], oh_d[t])
                    ohd_sb = ohdpool.tile([P, NE], FP8, tag="ohd")
                    nc.sync.dma_start(ohd_sb[:], ohd_d[t])

                    xlE = gpool.tile([P, NC, D], BF16, tag="xlE")
                    t_sb = tpool.tile([P, H, NE], BF16, tag="tsb")
                    plog = plogpool.tile([P, NC, 2], F32, tag="plog")

                    # ---- pass A: logits ----
                    for g in range(NG):
                        e0 = g * 512
                        nc.gpsimd.dma_gather(xlE[:, 4 * g:4 * (g + 1), :], Tfull[:],
                                             idx_sb[:, g, :], 512, 512, D,
                                             queue_num=(g % 2) * 2)
                        xlT = gtpool.tile([P, H, 512], BF16, tag="xlT")
                        nc.gpsimd.dma_gather(xlT[:], Tfull[:], idx_sb[:, g, :],
                                             512, 512, D, transpose=True,
                                             queue_num=(g % 2) * 2 + 1)
                        pm = pmpool.tile([P, H, 512], F32, tag="pm")
                        for h in range(H):
                            nc.tensor.matmul(pm[:, h, :],
                                             xr_sb[:, t, h * C:(h + 1) * C],
                                             ohd_sb[:, e0:e0 + 512],
                                             start=True, stop=False)
                        for h in range(H):
                            nc.tensor.matmul(pm[:, h, :], identbf[:],
                                             xlT[:, h, :], start=False, stop=True)
                        nc.scalar.activation(t_sb[:, :, e0:e0 + 512], pm[:],
                                             AF.Prelu, alpha=NEG_SLOPE)
                        for kl in range(4):
                            k = 4 * g + kl
                            for h in range(H):
                                nc.tensor.matmul(
                                    plog[:, k, :],
                                    t_sb[:, h, k * P:(k + 1) * P],
                                    attr_sb[:, l * 4 + h * 2:l * 4 + h * 2 + 2],
                                    start=(h == 0), stop=(h == 1))

                    # ---- exp + row form ----
                    evc = wpool.tile([P, NC * 2], BF16, tag="evc")
                    nc.scalar.activation(evc[:], plog[:], AF.Exp)
                    evT = plogpool.tile([NC * 2, P], BF16, tag="plog")
                    nc.tensor.transpose(evT[:], evc[:], identbf[:])
                    evrow = wpool.tile([NC * 2, P], BF16, tag="evrow")
                    nc.scalar.activation(evrow[:], evT[:], AF.Copy)

                    # ---- pass B: weighted aggregation ----
                    pseg = psegpool.tile([P, D + 2], F32, tag="pseg")
                    for g in range(NG):
                        evx = pevxpool.tile([P, 4, D], F32, tag="evx")
                        for kp in range(2):
                            k2 = 2 * g + kp
                            nc.tensor.matmul(evx[:, 2 * kp:2 * kp + 2, :], evrow[:],
                                             sel_sb[:, k2, :],
                                             start=True, stop=True)
                        sxl = sxlpool.tile([P, 4, D + 2], BF16, tag="sxl")
                        nc.vector.tensor_tensor(sxl[:, :, :D],
                                                xlE[:, 4 * g:4 * (g + 1), :],
                                                evx[:], ALU.mult)
                        nc.scalar.activation(sxl[:, :, D:D + 2],
                                             evc[:, 8 * g:8 * g + 8], AF.Copy)
                        for kl in range(4):
                            k = 4 * g + kl
                            nc.tensor.matmul(pseg[:], oh_sb[:, k, :],
                                             sxl[:, kl, :],
                                             start=(k == 0), stop=(k == NC - 1))

                    # ---- normalize + pool ----
                    rec = wpool.tile([P, 2], F32, tag="rec")
                    nc.vector.reciprocal(rec[:], pseg[:, D:D + 2])
                    hst = wpool.tile([P, D], BF16, tag="hst")
                    for h in range(H):
                        nc.vector.tensor_scalar(hst[:, h * C:(h + 1) * C],
                                                pseg[:, h * C:(h + 1) * C],
                                                rec[:, h:h + 1], None, ALU.mult)

                    Gt = wpool.tile([P, P], BF16, tag="Gt")
                    nc.vector.tensor_scalar(Gt[:], iotarow[:], bloc_sb[:, t:t + 1],
                                            None, ALU.is_equal)
                    nc.tensor.matmul(ppool[:, :D], Gt[:], hst[:],
                                     start=(t == 0), stop=(t == NT - 1 and l != 0))
                    if l == 0:
                        nc.tensor.matmul(ppool[:, D:D + 1], Gt[:], onescol[:],
                                         start=False, stop=(t == NT - 1))

                    # transpose h for next layer's table build
                    if l < 2:
                        for h in range(H):
                            ptr = pmiscpool.tile([P, P], BF16, tag="ptab")
                            nc.tensor.transpose(ptr[:], hst[:, h * C:(h + 1) * C],
                                                identbf[:])
                            nc.vector.tensor_copy(hT_sb[:, h, t * P:(t + 1) * P], ptr[:])

                nc.vector.tensor_copy(pool_sb[:, l * D:(l + 1) * D], ppool[:, :D])
                if l == 0:
                    nc.vector.tensor_copy(pool_sb[:, 768:769], ppool[:, D:D + 1])

            # ------------------------------------------------------------------
            # pooling allreduce + MLP
            nc.gpsimd.indirect_dma_start(
                out=poolb_in[:],
                out_offset=IndirectOffsetOnAxis(ap=poolidx_sb[:, :1], axis=0),
                in_=pool_sb[:],
                in_offset=None,
            )
            nc.gpsimd.collective_compute(
                "AllReduce", ALU.add, replica_groups=rg,
                ins=[poolb_in.opt()], outs=[poolb_out.opt()],
            )

            fW1_sb = [pp.tile([P, 768], F32, name=f"fW1_{kc}", tag=f"fW1_{kc}") for kc in range(7)]
            for kc in range(7):
                kr = min(P, PDA - kc * P)
                nc.sync.dma_start(fW1_sb[kc][:kr, :], fW1_d[kc * P:kc * P + kr, :])
            fb1_sb = pp.tile([P, 6], F32, tag="fb1")
            nc.sync.dma_start(fb1_sb[:], fb1_d[:])
            fW2_sb = pp.tile([P, 6], F32, tag="fW2")
            nc.sync.dma_start(fW2_sb[:], fW2_d[:])

            poolT = [pp.tile([P, max(G, P)], F32, name=f"poolT_{kc}", tag=f"poolT_{kc}") for kc in range(7)]
            for rt in range(0, G, P):
                rows = min(P, G - rt)
                prow = wpool.tile([P, PDA], F32, tag="prow")
                nc.sync.dma_start(prow[:rows, :], poolb_out[rt:rt + rows, :])
                for cb in range(7):
                    w = min(P, PDA - cb * P)
                    ptr2 = pmiscpool.tile([P, P], F32, tag="ptab")
                    nc.tensor.transpose(ptr2[:w, :rows], prow[:rows, cb * P:cb * P + w],
                                        identf[:rows, :rows])
                    nc.vector.tensor_copy(poolT[cb][:w, rt:rt + rows], ptr2[:w, :rows])

            h1_sb = [pp.tile([P, max(G, P)], F32, name=f"h1_{mo}", tag=f"h1_{mo}") for mo in range(6)]
            for mo in range(6):
                ph1 = pmiscpool.tile([P, max(G, P)], F32, tag="ptab")
                for kc in range(7):
                    kr = min(P, PDA - kc * P)
                    nc.tensor.matmul(ph1[:, :G], fW1_sb[kc][:kr, mo * P:(mo + 1) * P],
                                     poolT[kc][:kr, :G], start=(kc == 0), stop=(kc == 6))
                nc.scalar.activation(h1_sb[mo][:, :G], ph1[:, :G], AF.Relu,
                                     bias=fb1_sb[:, mo:mo + 1])

            py = pmiscpool.tile([1, max(G, P)], F32, tag="ppool")
            for mo in range(6):
                nc.tensor.matmul(py[:, :G], fW2_sb[:, mo:mo + 1], h1_sb[mo][:, :G],
                                 start=(mo == 0), stop=(mo == 5))
            ysb = wpool.tile([1, max(G, P)], F32, tag="ysb")
            nc.vector.tensor_copy(ysb[:, :G], py[:, :G])
            nc.sync.dma_start(y_d[:], ysb[:1, :G])

    nc.compile()
    return nc


# ----------------------------------------------------------------------------
# host preprocessing
# ----------------------------------------------------------------------------

def preprocess(inputs: dict, cfg: Cfg):
    n, g, ndev = cfg.n, cfg.g, cfg.ndev
    NPD, NT = cfg.npd, cfg.nt

    x = np.asarray(inputs["x"], np.float32)
    ei = np.asarray(inputs["edge_index"]).astype(np.int64)
    batch = np.asarray(inputs["batch"]).astype(np.int64)

    src = np.concatenate([ei[0], np.arange(n)])
    dst = np.concatenate([ei[1], np.arange(n)])
    order = np.argsort(dst, kind="stable")
    src, dst = src[order], dst[order]

    # per (dev, tile) edge lists
    tile_of = dst // P              # global dst tile id (NT per device)
    counts = np.bincount(tile_of, minlength=(n // P))
    nchunk = int(np.ceil(counts.max() / P))
    nchunk = ((nchunk + 3) // 4) * 4
    cfg.nchunk = nchunk
    NE = nchunk * P

    tile_start = np.zeros(n // P + 1, np.int64)
    np.cumsum(counts, out=tile_start[1:])

    def wrap_idx(a):  # [512] int16 -> [128, 32]
        w = a.reshape(-1, 16).T.copy()          # [16, 32]
        return np.tile(w, (8, 1))               # [128, 32]

    in_maps = []
    consts = {
        "identbf": np.eye(P, dtype=BF),
        "identf": np.eye(P, dtype=np.float32),
        "ident8": np.eye(P, dtype=F8),
        "iotarow": np.tile(np.arange(P, dtype=BF)[None, :], (P, 1)),
        "ones1p": np.ones((1, P), BF),
        "onescol": np.ones((P, 1), BF),
    }
    # att as zero-padded 2-col blocks per (layer, head): logit matmuls
    # accumulate both heads into the same [128, 2] psum columns.
    att_all = np.stack([np.asarray(inputs[f"att{l+1}"], np.float32) for l in range(3)])  # [3, H, C]
    attr = np.zeros((P, 12), np.float32)
    for l in range(3):
        for h in range(H):
            attr[:, l * 4 + h * 2 + h] = att_all[l, h]
    consts["attr"] = attr.astype(BF)
    # ev-row -> [128, 2, 256] expansion selector (chunk pairs, 512-wide):
    # sel[r, k2, kl*256 + c] = (r == 2*(2*k2+kl) + c//128)
    sel = np.zeros((nchunk * 2, nchunk // 2, 2 * D), np.float32)
    for k in range(nchunk):
        for h in range(H):
            sel[2 * k + h, k // 2, (k % 2) * D + h * C:(k % 2) * D + (h + 1) * C] = 1.0
    consts["sel"] = sel.astype(BF)

    b = [np.asarray(inputs[f"b{l+1}"], np.float32) for l in range(3)]
    wmats = {}
    for l in range(3):
        wmats[f"wl{l}"] = np.asarray(inputs[f"Wl{l+1}"], np.float32).astype(BF)
        wmats[f"wr{l}"] = np.asarray(inputs[f"Wr{l+1}"], np.float32).astype(BF)
    for l in (1, 2):
        wmats[f"rl{l}"] = (b[l - 1] @ np.asarray(inputs[f"Wl{l+1}"], np.float32))[None, :].astype(BF)
        wmats[f"rr{l}"] = (b[l - 1] @ np.asarray(inputs[f"Wr{l+1}"], np.float32))[None, :].astype(BF)

    b_all = np.concatenate(b)
    fW1 = np.asarray(inputs["fW1"], np.float32)
    fW1p = np.zeros((cfg.pda, 768), np.float32)
    fW1p[:768] = fW1
    fW1p[768] = b_all @ fW1
    fb1 = np.asarray(inputs["fb1"], np.float32).reshape(6, P).T.copy()
    fW2p = np.asarray(inputs["fW2"], np.float32).reshape(6, P).T.copy()
    fb2 = float(np.asarray(inputs["fb2"]).reshape(-1)[0])

    iota_p = np.arange(P)
    for dev in range(ndev):
        lo = dev * NPD
        g_lo = int(batch[lo])
        g_hi = int(batch[lo + NPD - 1])
        assert g_hi - g_lo + 1 <= P

        idxw = np.zeros((NT, P, nchunk // 4, 32), np.int16)
        oh = np.zeros((NT, P, nchunk, P), F8)
        ohd = np.zeros((NT, P, NE), F8)
        for t in range(NT):
            gt = dev * NT + t
            s, e = tile_start[gt], tile_start[gt + 1]
            cnt = e - s
            sp = np.zeros(NE, np.int64)
            sp[:cnt] = src[s:e]
            dl = np.full(NE, -1, np.int64)
            dl[:cnt] = dst[s:e] % P
            for gi in range(nchunk // 4):
                idxw[t, :, gi, :] = wrap_idx(sp[gi * 512:(gi + 1) * 512].astype(np.int16))
            # oh[t, e_in_chunk, k, d] = (dst_local(k*128+e) == d)  (seg lhsT)
            dl2 = dl.reshape(nchunk, P)                       # [k, e]
            oh_t = (dl2[:, :, None] == iota_p[None, None, :])  # [k, e, d]
            oh[t] = oh_t.transpose(1, 0, 2).astype(F8)
            # ohd[t, d, e] = (dst_local(e) == d)  (xr-broadcast moving)
            ohd[t] = (dl[None, :] == iota_p[:, None]).astype(F8)

        bloc = (batch[lo:lo + NPD].reshape(NT, P).T - g_lo).astype(np.float32)
        poolidx = np.arange(P, dtype=np.int32) + g_lo
        poolidx[poolidx > g_hi] = g
        m = {
            "xT": x[lo:lo + NPD].T.astype(BF),
            "idxw": idxw,
            "oh": oh,
            "ohd": ohd,
            "bloc": bloc,
            "poolidx": poolidx[:, None],
            "fW1p": fW1p, "fb1p": fb1, "fW2p": fW2p,
            **consts, **wmats,
        }
        in_maps.append(m)
    return in_maps, fb2


def kernel_impl(inputs, trace=False, trace_kwargs=None):
    cfg = Cfg(n=16384, g=256, ndev=8, nchunk=0)
    in_maps, fb2 = preprocess(inputs, cfg)
    nc = build_program(cfg, fb2)
    res = run_bass_kernel_spmd(nc, in_maps, core_ids=list(range(cfg.ndev)),
                               trace=trace, **(trace_kwargs or {}))
    y = np.asarray(res.results[0]["y"], np.float32).reshape(cfg.g, 1)
    return y + fb2, res


def kernel(**inputs) -> np.ndarray:
    y, _ = kernel_impl(inputs)
    return y


# revision 33
# speedup vs baseline: 1.0991x; 1.0991x over previous
"""Trainium2 Bass kernel for 3-layer GATv2 + sum-pool + MLP (nn_GAT_56977036148745).

Strategy (8 NeuronCores, SPMD), v2 — engine-balanced rewrite:
  - Nodes sharded into 8 contiguous slabs of 2048 (dst-sharding). Each core owns
    all edges whose destination lands in its slab (edges sorted by dst on host).
  - Per layer: each core computes its slab of the gather table T = h @ Wl and
    the local query table xr = h @ Wr; an AllGather assembles the full [N,256]
    bf16 table in DRAM.  Edge phase per 128-dst tile:
      pass A: per 512-edge group, PE broadcasts xr to edges via a host-built
        fp8 dst-onehot (moving operand), accumulates the channel-major gathered
        xl (identity matmul), ACT applies LeakyReLU, and per-chunk column-form
        matmuls against zero-padded att columns produce per-edge logits in one
        PSUM bank [128, 2*NC].
      between passes: one ACT exp gives per-edge softmax numerators (columns),
        one PE transpose + DVE copy gives them in row form.
      pass B: per chunk, a selector matmul expands ev rows to [128,256] PSUM,
        one group-wide DVE multiply scales the edge-major gathered xl, and one
        256-wide seg matmul per chunk against the host-built edge-onehot
        stationary accumulates numerators; a 2-wide matmul with the ev columns
        accumulates denominators in the same PSUM tile.
  - Sum-pool via graph-onehot matmuls, AllReduce of the pooled [G+1,772]
    bounce, MLP redundantly on every core (unchanged from v1).
Host preprocessing (sorting, padding, onehot construction, weight folding) is
not part of the measured device time.
"""

import sys

for _p in ("/opt/trn_rl_repo", "/root/.axon_site/_ro/trn_rl_repo"):
    if _p not in sys.path:
        sys.path.append(_p)

import numpy as np
import ml_dtypes

try:  # NTFF profiling hook shim (image's antenv lacks axon_hooks)
    import antenv.axon_hooks  # noqa: F401
except ImportError:
    import types as _types

    try:
        import trn_agent_boot.trn_boot as _tb
        _ntff_hook = _tb._ntff_profile_via_ctypes("/opt/axon/libaxon_pjrt.so")
    except Exception:
        _ntff_hook = None
    _m = _types.ModuleType("antenv.axon_hooks")
    _m.get_axon_ntff_profile_hook = lambda: _ntff_hook
    _m.set_axon_ntff_profile_hook = lambda h: None
    sys.modules["antenv.axon_hooks"] = _m

import concourse.bacc as bacc
import concourse.bass as bass
import concourse.mybir as mybir
import concourse.tile as tile
from concourse.bass import IndirectOffsetOnAxis
from concourse.bass_utils import run_bass_kernel_spmd

BF = ml_dtypes.bfloat16
F8 = ml_dtypes.float8_e4m3fn
F32 = mybir.dt.float32
BF16 = mybir.dt.bfloat16
FP8 = mybir.dt.float8e4
I16 = mybir.dt.int16
I32 = mybir.dt.int32

P = 128          # partitions / dst-tile size / edge-chunk size
H = 2            # heads
C = 128          # channels per head
D = H * C        # 256
AF = mybir.ActivationFunctionType
ALU = mybir.AluOpType
NEG_SLOPE = 0.2


class Cfg:
    def __init__(self, n, g, ndev, nchunk, in_ch=128):
        self.n = n                  # total nodes
        self.g = g                  # graphs
        self.ndev = ndev
        self.nchunk = nchunk        # edge chunks (of 128) per dst tile, mult of 4
        self.in_ch = in_ch          # layer-1 input channels
        self.npd = n // ndev        # nodes per device
        self.nt = self.npd // P     # dst/node tiles per device
        self.pda = 772              # padded pool dim (768 + cnt + pad)


# ----------------------------------------------------------------------------
# device program
# ----------------------------------------------------------------------------

def build_program(cfg: Cfg, fb2: float):
    nc = bacc.Bacc("TRN2", target_bir_lowering=False, debug=False,
                   num_devices=cfg.ndev, num_swdge_queues=4)
    NT, NC, G, NPD = cfg.nt, cfg.nchunk, cfg.g, cfg.npd
    NE = NC * P                      # padded edges per dst tile
    NG = NC // 4                     # 512-edge groups per tile
    KC1 = cfg.in_ch // P             # layer-1 K chunks (1)
    PDA = cfg.pda

    dt = nc.dram_tensor
    xT_d = dt("xT", [cfg.in_ch, NPD], BF16, kind="ExternalInput")
    wl_d = [dt(f"wl{l}", [cfg.in_ch if l == 0 else D, D], BF16, kind="ExternalInput") for l in range(3)]
    wr_d = [dt(f"wr{l}", [cfg.in_ch if l == 0 else D, D], BF16, kind="ExternalInput") for l in range(3)]
    rl_d = [None] + [dt(f"rl{l}", [1, D], BF16, kind="ExternalInput") for l in (1, 2)]
    rr_d = [None] + [dt(f"rr{l}", [1, D], BF16, kind="ExternalInput") for l in (1, 2)]
    attr_d = dt("attr", [P, 12], BF16, kind="ExternalInput")
    sel_d = dt("sel", [NC * 2, NC // 2, 2 * D], BF16, kind="ExternalInput")
    idxw_d = dt("idxw", [NT, P, NC // 4, 32], I16, kind="ExternalInput")
    oh_d = dt("oh", [NT, P, NC, P], FP8, kind="ExternalInput")
    ohd_d = dt("ohd", [NT, P, NE], FP8, kind="ExternalInput")
    ident8_d = dt("ident8", [P, P], FP8, kind="ExternalInput")
    bloc_d = dt("bloc", [P, NT], F32, kind="ExternalInput")
    poolidx_d = dt("poolidx", [P, 1], I32, kind="ExternalInput")
    fW1_d = dt("fW1p", [PDA, 768], F32, kind="ExternalInput")
    fb1_d = dt("fb1p", [P, 6], F32, kind="ExternalInput")
    fW2_d = dt("fW2p", [P, 6], F32, kind="ExternalInput")
    identbf_d = dt("identbf", [P, P], BF16, kind="ExternalInput")
    identf_d = dt("identf", [P, P], F32, kind="ExternalInput")
    iotarow_d = dt("iotarow", [P, P], BF16, kind="ExternalInput")
    ones1p_d = dt("ones1p", [1, P], BF16, kind="ExternalInput")
    onescol_d = dt("onescol", [P, 1], BF16, kind="ExternalInput")
    y_d = dt("y", [1, G], F32, kind="ExternalOutput")

    rg = [list(range(cfg.ndev))]

    with tile.TileContext(nc) as tc:
        with (
            tc.tile_pool(name="persist", bufs=1) as pp,
            tc.tile_pool(name="dram", bufs=2, space="DRAM") as dram,
            tc.tile_pool(name="oh", bufs=2) as ohpool,
            tc.tile_pool(name="ohd", bufs=2) as ohdpool,
            tc.tile_pool(name="gath", bufs=2) as gpool,
            tc.tile_pool(name="gt", bufs=6) as gtpool,
            tc.tile_pool(name="tsb", bufs=1) as tpool,
            tc.tile_pool(name="sxl", bufs=2) as sxlpool,
            tc.tile_pool(name="work", bufs=2) as wpool,
            tc.tile_pool(name="pm", bufs=1, space="PSUM") as pmpool,
            tc.tile_pool(name="plog", bufs=1, space="PSUM") as plogpool,
            tc.tile_pool(name="pevx", bufs=1, space="PSUM") as pevxpool,
            tc.tile_pool(name="pseg", bufs=1, space="PSUM") as psegpool,
            tc.tile_pool(name="pmisc", bufs=1, space="PSUM") as pmiscpool,
        ):
            # ---- persistent SBUF ----
            identbf = pp.tile([P, P], BF16, tag="identbf")
            identf = pp.tile([P, P], F32, tag="identf")
            iotarow = pp.tile([P, P], BF16, tag="iotarow")
            ones1p = pp.tile([1, P], BF16, tag="ones1p")
            onescol = pp.tile([P, 1], BF16, tag="onescol")
            attr_sb = pp.tile([P, 12], BF16, tag="attr")
            sel_sb = pp.tile([NC * 2, NC // 2, 2 * D], BF16, tag="sel")
            ident8 = pp.tile([P, P], FP8, tag="ident8")
            xT_sb = pp.tile([cfg.in_ch, NPD], BF16, tag="xT")
            hT_sb = pp.tile([P, 2, NPD], BF16, tag="hT")
            xr_sb = pp.tile([P, NT, D], BF16, tag="xr")
            pool_sb = pp.tile([P, PDA], F32, tag="pool")
            bloc_sb = pp.tile([P, NT], F32, tag="bloc")
            poolidx_sb = pp.tile([P, 1], I32, tag="poolidx")
            wl_sb = [pp.tile([P, (cfg.in_ch if l == 0 else D) // P, D], BF16, name=f"wl{l}", tag=f"wl{l}") for l in range(3)]
            wr_sb = [pp.tile([P, (cfg.in_ch if l == 0 else D) // P, D], BF16, name=f"wr{l}", tag=f"wr{l}") for l in range(3)]
            rl_sb = [None, pp.tile([1, D], BF16, name="rl1", tag="rl1"), pp.tile([1, D], BF16, name="rl2", tag="rl2")]
            rr_sb = [None, pp.tile([1, D], BF16, name="rr1", tag="rr1"), pp.tile([1, D], BF16, name="rr2", tag="rr2")]
            zero_sb = pp.tile([P, PDA], F32, tag="zero")

            for sb, d in ((identbf, identbf_d), (identf, identf_d),
                          (iotarow, iotarow_d), (ones1p, ones1p_d),
                          (onescol, onescol_d), (attr_sb, attr_d),
                          (sel_sb, sel_d), (xT_sb, xT_d), (bloc_sb, bloc_d),
                          (poolidx_sb, poolidx_d), (ident8, ident8_d)):
                nc.sync.dma_start(sb[:], d[:])
            for l in range(3):
                nc.sync.dma_start(wl_sb[l][:], wl_d[l].ap().rearrange("(k p) d -> p k d", p=P))
                nc.sync.dma_start(wr_sb[l][:], wr_d[l].ap().rearrange("(k p) d -> p k d", p=P))
                if l > 0:
                    nc.sync.dma_start(rl_sb[l][:], rl_d[l][:])
                    nc.sync.dma_start(rr_sb[l][:], rr_d[l][:])
            nc.vector.memset(pool_sb[:], 0.0)
            nc.vector.memset(zero_sb[:], 0.0)

            # pool bounce (zeroed before scatter)
            poolb_in = dram.tile([G + 1, PDA], F32, tag="poolb_in")
            poolb_out = dram.tile([G + 1, PDA], F32, tag="poolb_out")
            for r0 in range(0, G + 1, P):
                rows = min(P, G + 1 - r0)
                nc.sync.dma_start(poolb_in[r0:r0 + rows, :], zero_sb[:rows, :])

            # ------------------------------------------------------------------
            for l in range(3):
                kcs = KC1 if l == 0 else 2
                src_sb = xT_sb if l == 0 else hT_sb

                def src_lhsT(kc, nt):
                    if l == 0:
                        return src_sb[:, nt * P:(nt + 1) * P]
                    return src_sb[:, kc, nt * P:(nt + 1) * P]

                # ---- table slab + XR slab ----
                slab = dram.tile([NPD, D], BF16, tag="slab")
                Tfull = dram.tile([cfg.n, D], BF16, tag="Tfull")
                for nt in range(NT):
                    ptab = pmiscpool.tile([P, D], F32, tag="ptab")
                    for kc in range(kcs):
                        nc.tensor.matmul(ptab[:], src_lhsT(kc, nt), wl_sb[l][:, kc, :],
                                         start=(kc == 0), stop=(kc == kcs - 1 and l == 0))
                    if l > 0:
                        nc.tensor.matmul(ptab[:], ones1p[:], rl_sb[l][:], start=False, stop=True)
                    tab = wpool.tile([P, D], BF16, tag="tab")
                    nc.vector.tensor_copy(tab[:], ptab[:])
                    nc.sync.dma_start(slab[nt * P:(nt + 1) * P, :], tab[:])

                    pxr = pmiscpool.tile([P, D], F32, tag="ptab")
                    for kc in range(kcs):
                        nc.tensor.matmul(pxr[:], src_lhsT(kc, nt), wr_sb[l][:, kc, :],
                                         start=(kc == 0), stop=(kc == kcs - 1 and l == 0))
                    if l > 0:
                        nc.tensor.matmul(pxr[:], ones1p[:], rr_sb[l][:], start=False, stop=True)
                    nc.vector.tensor_copy(xr_sb[:, nt, :], pxr[:])

                nc.gpsimd.collective_compute(
                    "AllGather", ALU.bypass, replica_groups=rg,
                    ins=[slab.opt()], outs=[Tfull.opt()],
                )

                # ---- edge phase ----
                ppool = pmiscpool.tile([P, D + 1], F32, tag="ppool")
                idxall_sb = wpool.tile([P, NT, NC // 4, 32], I16, tag="idxall")
                nc.sync.dma_start(idxall_sb[:], idxw_d.ap().rearrange("t p g w -> p t g w"))
                for t in range(NT):
                    idx_sb = idxall_sb[:, t]
                    oh_sb = ohpool.tile([P, NC, P], FP8, tag="oh")
                    nc.sync.dma_start(oh_sb[:], oh_d[t])
                    ohd_sb = ohdpool.tile([P, NE], FP8, tag="ohd")
                    nc.sync.dma_start(ohd_sb[:], ohd_d[t])

                    xlE = gpool.tile([P, NC, D], BF16, tag="xlE")
                    t_sb = tpool.tile([P, H, NE], BF16, tag="tsb")
                    plog = plogpool.tile([P, NC, 2], F32, tag="plog")

                    # ---- pass A: logits ----
                    for g in range(NG):
                        e0 = g * 512
                        nc.gpsimd.dma_gather(xlE[:, 4 * g:4 * (g + 1), :], Tfull[:],
                                             idx_sb[:, g, :], 512, 512, D,
                                             queue_num=(g % 2) * 2)
                        xlT = gtpool.tile([P, H, 512], BF16, tag="xlT")
                        nc.gpsimd.dma_gather(xlT[:], Tfull[:], idx_sb[:, g, :],
                                             512, 512, D, transpose=True,
                                             queue_num=(g % 2) * 2 + 1)
                        pm = pmpool.tile([P, H, 512], F32, tag="pm")
                        for h in range(H):
                            nc.tensor.matmul(pm[:, h, :],
                                             xr_sb[:, t, h * C:(h + 1) * C],
                                             ohd_sb[:, e0:e0 + 512],
                                             start=True, stop=False)
                            nc.tensor.matmul(pm[:, h, :], identbf[:],
                                             xlT[:, h, :], start=False, stop=True)
                            nc.scalar.activation(t_sb[:, h, e0:e0 + 512],
                                                 pm[:, h, :], AF.Prelu,
                                                 alpha=NEG_SLOPE)
                        for kl in range(4):
                            k = 4 * g + kl
                            for h in range(H):
                                nc.tensor.matmul(
                                    plog[:, k, :],
                                    t_sb[:, h, k * P:(k + 1) * P],
                                    attr_sb[:, l * 4 + h * 2:l * 4 + h * 2 + 2],
                                    start=(h == 0), stop=(h == 1))

                    # ---- exp + row form ----
                    evc = wpool.tile([P, NC * 2], BF16, tag="evc")
                    nc.scalar.activation(evc[:], plog[:], AF.Exp)
                    evT = plogpool.tile([NC * 2, P], BF16, tag="plog")
                    nc.tensor.transpose(evT[:], evc[:], identbf[:])
                    evrow = wpool.tile([NC * 2, P], BF16, tag="evrow")
                    nc.scalar.activation(evrow[:], evT[:], AF.Copy)

                    # ---- pass B: weighted aggregation ----
                    pseg = psegpool.tile([P, D + 2], F32, tag="pseg")
                    for g in range(NG):
                        evx = pevxpool.tile([P, 4, D], F32, tag="evx")
                        for kp in range(2):
                            k2 = 2 * g + kp
                            nc.tensor.matmul(evx[:, 2 * kp:2 * kp + 2, :], evrow[:],
                                             sel_sb[:, k2, :],
                                             start=True, stop=True)
                        sxl = sxlpool.tile([P, 4, D + 2], BF16, tag="sxl")
                        nc.vector.tensor_tensor(sxl[:, :, :D],
                                                xlE[:, 4 * g:4 * (g + 1), :],
                                                evx[:], ALU.mult)
                        nc.scalar.activation(sxl[:, :, D:D + 2],
                                             evc[:, 8 * g:8 * g + 8], AF.Copy)
                        for kl in range(4):
                            k = 4 * g + kl
                            nc.tensor.matmul(pseg[:], oh_sb[:, k, :],
                                             sxl[:, kl, :],
                                             start=(k == 0), stop=(k == NC - 1))

                    # ---- normalize + pool ----
                    rec = wpool.tile([P, 2], F32, tag="rec")
                    nc.vector.reciprocal(rec[:], pseg[:, D:D + 2])
                    hst = wpool.tile([P, D], BF16, tag="hst")
                    for h in range(H):
                        nc.vector.tensor_scalar(hst[:, h * C:(h + 1) * C],
                                                pseg[:, h * C:(h + 1) * C],
                                                rec[:, h:h + 1], None, ALU.mult)

                    Gt = wpool.tile([P, P], BF16, tag="Gt")
                    nc.vector.tensor_scalar(Gt[:], iotarow[:], bloc_sb[:, t:t + 1],
                                            None, ALU.is_equal)
                    nc.tensor.matmul(ppool[:, :D], Gt[:], hst[:],
                                     start=(t == 0), stop=(t == NT - 1 and l != 0))
                    if l == 0:
                        nc.tensor.matmul(ppool[:, D:D + 1], Gt[:], onescol[:],
                                         start=False, stop=(t == NT - 1))

                    # transpose h for next layer's table build
                    if l < 2:
                        for h in range(H):
                            ptr = pmiscpool.tile([P, P], BF16, tag="ptab")
                            nc.tensor.transpose(ptr[:], hst[:, h * C:(h + 1) * C],
                                                identbf[:])
                            nc.vector.tensor_copy(hT_sb[:, h, t * P:(t + 1) * P], ptr[:])

                nc.vector.tensor_copy(pool_sb[:, l * D:(l + 1) * D], ppool[:, :D])
                if l == 0:
                    nc.vector.tensor_copy(pool_sb[:, 768:769], ppool[:, D:D + 1])

            # ------------------------------------------------------------------
            # pooling allreduce + MLP
            nc.gpsimd.indirect_dma_start(
                out=poolb_in[:],
                out_offset=IndirectOffsetOnAxis(ap=poolidx_sb[:, :1], axis=0),
                in_=pool_sb[:],
                in_offset=None,
            )
            nc.gpsimd.collective_compute(
                "AllReduce", ALU.add, replica_groups=rg,
                ins=[poolb_in.opt()], outs=[poolb_out.opt()],
            )

            fW1_sb = [pp.tile([P, 768], F32, name=f"fW1_{kc}", tag=f"fW1_{kc}") for kc in range(7)]
            for kc in range(7):
                kr = min(P, PDA - kc * P)
                nc.sync.dma_start(fW1_sb[kc][:kr, :], fW1_d[kc * P:kc * P + kr, :])
            fb1_sb = pp.tile([P, 6], F32, tag="fb1")
            nc.sync.dma_start(fb1_sb[:], fb1_d[:])
            fW2_sb = pp.tile([P, 6], F32, tag="fW2")
            nc.sync.dma_start(fW2_sb[:], fW2_d[:])

            poolT = [pp.tile([P, max(G, P)], F32, name=f"poolT_{kc}", tag=f"poolT_{kc}") for kc in range(7)]
            for rt in range(0, G, P):
                rows = min(P, G - rt)
                prow = wpool.tile([P, PDA], F32, tag="prow")
                nc.sync.dma_start(prow[:rows, :], poolb_out[rt:rt + rows, :])
                for cb in range(7):
                    w = min(P, PDA - cb * P)
                    ptr2 = pmiscpool.tile([P, P], F32, tag="ptab")
                    nc.tensor.transpose(ptr2[:w, :rows], prow[:rows, cb * P:cb * P + w],
                                        identf[:rows, :rows])
                    nc.vector.tensor_copy(poolT[cb][:w, rt:rt + rows], ptr2[:w, :rows])

            h1_sb = [pp.tile([P, max(G, P)], F32, name=f"h1_{mo}", tag=f"h1_{mo}") for mo in range(6)]
            for mo in range(6):
                ph1 = pmiscpool.tile([P, max(G, P)], F32, tag="ptab")
                for kc in range(7):
                    kr = min(P, PDA - kc * P)
                    nc.tensor.matmul(ph1[:, :G], fW1_sb[kc][:kr, mo * P:(mo + 1) * P],
                                     poolT[kc][:kr, :G], start=(kc == 0), stop=(kc == 6))
                nc.scalar.activation(h1_sb[mo][:, :G], ph1[:, :G], AF.Relu,
                                     bias=fb1_sb[:, mo:mo + 1])

            py = pmiscpool.tile([1, max(G, P)], F32, tag="ppool")
            for mo in range(6):
                nc.tensor.matmul(py[:, :G], fW2_sb[:, mo:mo + 1], h1_sb[mo][:, :G],
                                 start=(mo == 0), stop=(mo == 5))
            ysb = wpool.tile([1, max(G, P)], F32, tag="ysb")
            nc.vector.tensor_copy(ysb[:, :G], py[:, :G])
            nc.sync.dma_start(y_d[:], ysb[:1, :G])

    nc.compile()
    return nc


# ----------------------------------------------------------------------------
# host preprocessing
# ----------------------------------------------------------------------------

def preprocess(inputs: dict, cfg: Cfg):
    n, g, ndev = cfg.n, cfg.g, cfg.ndev
    NPD, NT = cfg.npd, cfg.nt

    x = np.asarray(inputs["x"], np.float32)
    ei = np.asarray(inputs["edge_index"]).astype(np.int64)
    batch = np.asarray(inputs["batch"]).astype(np.int64)

    src = np.concatenate([ei[0], np.arange(n)])
    dst = np.concatenate([ei[1], np.arange(n)])
    order = np.argsort(dst, kind="stable")
    src, dst = src[order], dst[order]

    # per (dev, tile) edge lists
    tile_of = dst // P              # global dst tile id (NT per device)
    counts = np.bincount(tile_of, minlength=(n // P))
    nchunk = int(np.ceil(counts.max() / P))
    nchunk = ((nchunk + 3) // 4) * 4
    cfg.nchunk = nchunk
    NE = nchunk * P

    tile_start = np.zeros(n // P + 1, np.int64)
    np.cumsum(counts, out=tile_start[1:])

    def wrap_idx(a):  # [512] int16 -> [128, 32]
        w = a.reshape(-1, 16).T.copy()          # [16, 32]
        return np.tile(w, (8, 1))               # [128, 32]

    in_maps = []
    consts = {
        "identbf": np.eye(P, dtype=BF),
        "identf": np.eye(P, dtype=np.float32),
        "ident8": np.eye(P, dtype=F8),
        "iotarow": np.tile(np.arange(P, dtype=BF)[None, :], (P, 1)),
        "ones1p": np.ones((1, P), BF),
        "onescol": np.ones((P, 1), BF),
    }
    # att as zero-padded 2-col blocks per (layer, head): logit matmuls
    # accumulate both heads into the same [128, 2] psum columns.
    att_all = np.stack([np.asarray(inputs[f"att{l+1}"], np.float32) for l in range(3)])  # [3, H, C]
    attr = np.zeros((P, 12), np.float32)
    for l in range(3):
        for h in range(H):
            attr[:, l * 4 + h * 2 + h] = att_all[l, h]
    consts["attr"] = attr.astype(BF)
    # ev-row -> [128, 2, 256] expansion selector (chunk pairs, 512-wide):
    # sel[r, k2, kl*256 + c] = (r == 2*(2*k2+kl) + c//128)
    sel = np.zeros((nchunk * 2, nchunk // 2, 2 * D), np.float32)
    for k in range(nchunk):
        for h in range(H):
            sel[2 * k + h, k // 2, (k % 2) * D + h * C:(k % 2) * D + (h + 1) * C] = 1.0
    consts["sel"] = sel.astype(BF)

    b = [np.asarray(inputs[f"b{l+1}"], np.float32) for l in range(3)]
    wmats = {}
    for l in range(3):
        wmats[f"wl{l}"] = np.asarray(inputs[f"Wl{l+1}"], np.float32).astype(BF)
        wmats[f"wr{l}"] = np.asarray(inputs[f"Wr{l+1}"], np.float32).astype(BF)
    for l in (1, 2):
        wmats[f"rl{l}"] = (b[l - 1] @ np.asarray(inputs[f"Wl{l+1}"], np.float32))[None, :].astype(BF)
        wmats[f"rr{l}"] = (b[l - 1] @ np.asarray(inputs[f"Wr{l+1}"], np.float32))[None, :].astype(BF)

    b_all = np.concatenate(b)
    fW1 = np.asarray(inputs["fW1"], np.float32)
    fW1p = np.zeros((cfg.pda, 768), np.float32)
    fW1p[:768] = fW1
    fW1p[768] = b_all @ fW1
    fb1 = np.asarray(inputs["fb1"], np.float32).reshape(6, P).T.copy()
    fW2p = np.asarray(inputs["fW2"], np.float32).reshape(6, P).T.copy()
    fb2 = float(np.asarray(inputs["fb2"]).reshape(-1)[0])

    iota_p = np.arange(P)
    for dev in range(ndev):
        lo = dev * NPD
        g_lo = int(batch[lo])
        g_hi = int(batch[lo + NPD - 1])
        assert g_hi - g_lo + 1 <= P

        idxw = np.zeros((NT, P, nchunk // 4, 32), np.int16)
        oh = np.zeros((NT, P, nchunk, P), F8)
        ohd = np.zeros((NT, P, NE), F8)
        for t in range(NT):
            gt = dev * NT + t
            s, e = tile_start[gt], tile_start[gt + 1]
            cnt = e - s
            sp = np.zeros(NE, np.int64)
            sp[:cnt] = src[s:e]
            dl = np.full(NE, -1, np.int64)
            dl[:cnt] = dst[s:e] % P
            for gi in range(nchunk // 4):
                idxw[t, :, gi, :] = wrap_idx(sp[gi * 512:(gi + 1) * 512].astype(np.int16))
            # oh[t, e_in_chunk, k, d] = (dst_local(k*128+e) == d)  (seg lhsT)
            dl2 = dl.reshape(nchunk, P)                       # [k, e]
            oh_t = (dl2[:, :, None] == iota_p[None, None, :])  # [k, e, d]
            oh[t] = oh_t.transpose(1, 0, 2).astype(F8)
            # ohd[t, d, e] = (dst_local(e) == d)  (xr-broadcast moving)
            ohd[t] = (dl[None, :] == iota_p[:, None]).astype(F8)

        bloc = (batch[lo:lo + NPD].reshape(NT, P).T - g_lo).astype(np.float32)
        poolidx = np.arange(P, dtype=np.int32) + g_lo
        poolidx[poolidx > g_hi] = g
        m = {
            "xT": x[lo:lo + NPD].T.astype(BF),
            "idxw": idxw,
            "oh": oh,
            "ohd": ohd,
            "bloc": bloc,
            "poolidx": poolidx[:, None],
            "fW1p": fW1p, "fb1p": fb1, "fW2p": fW2p,
            **consts, **wmats,
        }
        in_maps.append(m)
    return in_maps, fb2


def kernel_impl(inputs, trace=False, trace_kwargs=None):
    cfg = Cfg(n=16384, g=256, ndev=8, nchunk=0)
    in_maps, fb2 = preprocess(inputs, cfg)
    nc = build_program(cfg, fb2)
    res = run_bass_kernel_spmd(nc, in_maps, core_ids=list(range(cfg.ndev)),
                               trace=trace, **(trace_kwargs or {}))
    y = np.asarray(res.results[0]["y"], np.float32).reshape(cfg.g, 1)
    return y + fb2, res


def kernel(**inputs) -> np.ndarray:
    y, _ = kernel_impl(inputs)
    return y


# revision 39
# speedup vs baseline: 1.1045x; 1.0049x over previous
"""Trainium2 Bass kernel for 3-layer GATv2 + sum-pool + MLP (nn_GAT_56977036148745).

Strategy (8 NeuronCores, SPMD), v2 — engine-balanced rewrite:
  - Nodes sharded into 8 contiguous slabs of 2048 (dst-sharding). Each core owns
    all edges whose destination lands in its slab (edges sorted by dst on host).
  - Per layer: each core computes its slab of the gather table T = h @ Wl and
    the local query table xr = h @ Wr; an AllGather assembles the full [N,256]
    bf16 table in DRAM.  Edge phase per 128-dst tile:
      pass A: per 512-edge group, PE broadcasts xr to edges via a host-built
        fp8 dst-onehot (moving operand), accumulates the channel-major gathered
        xl (identity matmul), ACT applies LeakyReLU, and per-chunk column-form
        matmuls against zero-padded att columns produce per-edge logits in one
        PSUM bank [128, 2*NC].
      between passes: one ACT exp gives per-edge softmax numerators (columns),
        one PE transpose + DVE copy gives them in row form.
      pass B: per chunk, a selector matmul expands ev rows to [128,256] PSUM,
        one group-wide DVE multiply scales the edge-major gathered xl, and one
        256-wide seg matmul per chunk against the host-built edge-onehot
        stationary accumulates numerators; a 2-wide matmul with the ev columns
        accumulates denominators in the same PSUM tile.
  - Sum-pool via graph-onehot matmuls, AllReduce of the pooled [G+1,772]
    bounce, MLP redundantly on every core (unchanged from v1).
Host preprocessing (sorting, padding, onehot construction, weight folding) is
not part of the measured device time.
"""

import sys

for _p in ("/opt/trn_rl_repo", "/root/.axon_site/_ro/trn_rl_repo"):
    if _p not in sys.path:
        sys.path.append(_p)

import numpy as np
import ml_dtypes

try:  # NTFF profiling hook shim (image's antenv lacks axon_hooks)
    import antenv.axon_hooks  # noqa: F401
except ImportError:
    import types as _types

    try:
        import trn_agent_boot.trn_boot as _tb
        _ntff_hook = _tb._ntff_profile_via_ctypes("/opt/axon/libaxon_pjrt.so")
    except Exception:
        _ntff_hook = None
    _m = _types.ModuleType("antenv.axon_hooks")
    _m.get_axon_ntff_profile_hook = lambda: _ntff_hook
    _m.set_axon_ntff_profile_hook = lambda h: None
    sys.modules["antenv.axon_hooks"] = _m

import concourse.bacc as bacc
import concourse.bass as bass
import concourse.mybir as mybir
import concourse.tile as tile
from concourse.bass import IndirectOffsetOnAxis
from concourse.bass_utils import run_bass_kernel_spmd

BF = ml_dtypes.bfloat16
F8 = ml_dtypes.float8_e4m3fn
F32 = mybir.dt.float32
BF16 = mybir.dt.bfloat16
FP8 = mybir.dt.float8e4
I16 = mybir.dt.int16
I32 = mybir.dt.int32

P = 128          # partitions / dst-tile size / edge-chunk size
H = 2            # heads
C = 128          # channels per head
D = H * C        # 256
AF = mybir.ActivationFunctionType
ALU = mybir.AluOpType
NEG_SLOPE = 0.2


class Cfg:
    def __init__(self, n, g, ndev, nchunk, in_ch=128):
        self.n = n                  # total nodes
        self.g = g                  # graphs
        self.ndev = ndev
        self.nchunk = nchunk        # edge chunks (of 128) per dst tile, mult of 4
        self.in_ch = in_ch          # layer-1 input channels
        self.npd = n // ndev        # nodes per device
        self.nt = self.npd // P     # dst/node tiles per device
        self.pda = 772              # padded pool dim (768 + cnt + pad)


# ----------------------------------------------------------------------------
# device program
# ----------------------------------------------------------------------------

def build_program(cfg: Cfg, fb2: float):
    nc = bacc.Bacc("TRN2", target_bir_lowering=False, debug=False,
                   num_devices=cfg.ndev, num_swdge_queues=4)
    NT, NC, G, NPD = cfg.nt, cfg.nchunk, cfg.g, cfg.npd
    NE = NC * P                      # padded edges per dst tile
    NG = NC // 4                     # 512-edge groups per tile
    KC1 = cfg.in_ch // P             # layer-1 K chunks (1)
    PDA = cfg.pda

    dt = nc.dram_tensor
    xT_d = dt("xT", [cfg.in_ch, NPD], BF16, kind="ExternalInput")
    wl_d = [dt(f"wl{l}", [cfg.in_ch if l == 0 else D, D], BF16, kind="ExternalInput") for l in range(3)]
    wr_d = [dt(f"wr{l}", [cfg.in_ch if l == 0 else D, D], BF16, kind="ExternalInput") for l in range(3)]
    rl_d = [None] + [dt(f"rl{l}", [1, D], BF16, kind="ExternalInput") for l in (1, 2)]
    rr_d = [None] + [dt(f"rr{l}", [1, D], BF16, kind="ExternalInput") for l in (1, 2)]
    attr_d = dt("attr", [P, 12], BF16, kind="ExternalInput")
    sel_d = dt("sel", [NC * 2, NC // 2, 2 * D], BF16, kind="ExternalInput")
    idxw_d = dt("idxw", [NT, P, NC // 4, 32], I16, kind="ExternalInput")
    oh_d = dt("oh", [NT, P, NC, P], FP8, kind="ExternalInput")
    ohd_d = dt("ohd", [NT, P, NE], FP8, kind="ExternalInput")
    ident8_d = dt("ident8", [P, P], FP8, kind="ExternalInput")
    bloc_d = dt("bloc", [P, NT], F32, kind="ExternalInput")
    poolidx_d = dt("poolidx", [P, 1], I32, kind="ExternalInput")
    fW1_d = dt("fW1p", [PDA, 768], F32, kind="ExternalInput")
    fb1_d = dt("fb1p", [P, 6], F32, kind="ExternalInput")
    fW2_d = dt("fW2p", [P, 6], F32, kind="ExternalInput")
    identbf_d = dt("identbf", [P, P], BF16, kind="ExternalInput")
    identf_d = dt("identf", [P, P], F32, kind="ExternalInput")
    iotarow_d = dt("iotarow", [P, P], BF16, kind="ExternalInput")
    ones1p_d = dt("ones1p", [1, P], BF16, kind="ExternalInput")
    onescol_d = dt("onescol", [P, 1], BF16, kind="ExternalInput")
    y_d = dt("y", [1, G], F32, kind="ExternalOutput")

    rg = [list(range(cfg.ndev))]

    with tile.TileContext(nc) as tc:
        with (
            tc.tile_pool(name="persist", bufs=1) as pp,
            tc.tile_pool(name="dram", bufs=2, space="DRAM") as dram,
            tc.tile_pool(name="oh", bufs=2) as ohpool,
            tc.tile_pool(name="ohd", bufs=2) as ohdpool,
            tc.tile_pool(name="gath", bufs=2) as gpool,
            tc.tile_pool(name="gt", bufs=6) as gtpool,
            tc.tile_pool(name="tsb", bufs=1) as tpool,
            tc.tile_pool(name="sxl", bufs=2) as sxlpool,
            tc.tile_pool(name="work", bufs=2) as wpool,
            tc.tile_pool(name="pm", bufs=1, space="PSUM") as pmpool,
            tc.tile_pool(name="plog", bufs=1, space="PSUM") as plogpool,
            tc.tile_pool(name="pevx", bufs=1, space="PSUM") as pevxpool,
            tc.tile_pool(name="pseg", bufs=1, space="PSUM") as psegpool,
            tc.tile_pool(name="pmisc", bufs=1, space="PSUM") as pmiscpool,
        ):
            # ---- persistent SBUF ----
            identbf = pp.tile([P, P], BF16, tag="identbf")
            identf = pp.tile([P, P], F32, tag="identf")
            iotarow = pp.tile([P, P], BF16, tag="iotarow")
            ones1p = pp.tile([1, P], BF16, tag="ones1p")
            onescol = pp.tile([P, 1], BF16, tag="onescol")
            attr_sb = pp.tile([P, 12], BF16, tag="attr")
            sel_sb = pp.tile([NC * 2, NC // 2, 2 * D], BF16, tag="sel")
            ident8 = pp.tile([P, P], FP8, tag="ident8")
            xT_sb = pp.tile([cfg.in_ch, NPD], BF16, tag="xT")
            hT_sb = pp.tile([P, 2, NPD], BF16, tag="hT")
            xr_sb = pp.tile([P, NT, D], BF16, tag="xr")
            pool_sb = pp.tile([P, PDA], F32, tag="pool")
            bloc_sb = pp.tile([P, NT], F32, tag="bloc")
            poolidx_sb = pp.tile([P, 1], I32, tag="poolidx")
            wl_sb = [pp.tile([P, (cfg.in_ch if l == 0 else D) // P, D], BF16, name=f"wl{l}", tag=f"wl{l}") for l in range(3)]
            wr_sb = [pp.tile([P, (cfg.in_ch if l == 0 else D) // P, D], BF16, name=f"wr{l}", tag=f"wr{l}") for l in range(3)]
            rl_sb = [None, pp.tile([1, D], BF16, name="rl1", tag="rl1"), pp.tile([1, D], BF16, name="rl2", tag="rl2")]
            rr_sb = [None, pp.tile([1, D], BF16, name="rr1", tag="rr1"), pp.tile([1, D], BF16, name="rr2", tag="rr2")]
            zero_sb = pp.tile([P, PDA], F32, tag="zero")

            for sb, d in ((identbf, identbf_d), (identf, identf_d),
                          (iotarow, iotarow_d), (ones1p, ones1p_d),
                          (onescol, onescol_d), (attr_sb, attr_d),
                          (sel_sb, sel_d), (xT_sb, xT_d), (bloc_sb, bloc_d),
                          (poolidx_sb, poolidx_d), (ident8, ident8_d)):
                nc.sync.dma_start(sb[:], d[:])
            for l in range(3):
                nc.sync.dma_start(wl_sb[l][:], wl_d[l].ap().rearrange("(k p) d -> p k d", p=P))
                nc.sync.dma_start(wr_sb[l][:], wr_d[l].ap().rearrange("(k p) d -> p k d", p=P))
                if l > 0:
                    nc.sync.dma_start(rl_sb[l][:], rl_d[l][:])
                    nc.sync.dma_start(rr_sb[l][:], rr_d[l][:])
            nc.vector.memset(pool_sb[:], 0.0)
            nc.vector.memset(zero_sb[:], 0.0)

            # pool bounce (zeroed before scatter)
            poolb_in = dram.tile([G + 1, PDA], F32, tag="poolb_in")
            poolb_out = dram.tile([G + 1, PDA], F32, tag="poolb_out")
            for r0 in range(0, G + 1, P):
                rows = min(P, G + 1 - r0)
                nc.sync.dma_start(poolb_in[r0:r0 + rows, :], zero_sb[:rows, :])

            # ------------------------------------------------------------------
            def build_table_tile(l, nt, slab):
                """Emit layer-l table rows + xr rows for node tile nt.

                l==0 reads xT_sb; l>0 reads hT_sb (the transposed previous-
                layer output, written per tile during the previous edge phase).
                """
                kcs = KC1 if l == 0 else 2

                def src_lhsT(kc):
                    if l == 0:
                        return xT_sb[:, nt * P:(nt + 1) * P]
                    return hT_sb[:, kc, nt * P:(nt + 1) * P]

                ptab = pmiscpool.tile([P, D], F32, tag="ptab")
                for kc in range(kcs):
                    nc.tensor.matmul(ptab[:], src_lhsT(kc), wl_sb[l][:, kc, :],
                                     start=(kc == 0), stop=(kc == kcs - 1 and l == 0))
                if l > 0:
                    nc.tensor.matmul(ptab[:], ones1p[:], rl_sb[l][:], start=False, stop=True)
                tab = wpool.tile([P, D], BF16, tag="tab")
                nc.vector.tensor_copy(tab[:], ptab[:])
                nc.sync.dma_start(slab[nt * P:(nt + 1) * P, :], tab[:])

                pxr = pmiscpool.tile([P, D], F32, tag="ptab")
                for kc in range(kcs):
                    nc.tensor.matmul(pxr[:], src_lhsT(kc), wr_sb[l][:, kc, :],
                                     start=(kc == 0), stop=(kc == kcs - 1 and l == 0))
                if l > 0:
                    nc.tensor.matmul(pxr[:], ones1p[:], rr_sb[l][:], start=False, stop=True)
                nc.vector.tensor_copy(xr_sb[:, nt, :], pxr[:])

            # MLP weights: prefetch while the edge phases run
            fW1_sb = [pp.tile([P, 768], F32, name=f"fW1_{kc}", tag=f"fW1_{kc}") for kc in range(7)]
            for kc in range(7):
                kr = min(P, PDA - kc * P)
                nc.sync.dma_start(fW1_sb[kc][:kr, :], fW1_d[kc * P:kc * P + kr, :])
            fb1_sb = pp.tile([P, 6], F32, tag="fb1")
            nc.sync.dma_start(fb1_sb[:], fb1_d[:])
            fW2_sb = pp.tile([P, 6], F32, tag="fW2")
            nc.sync.dma_start(fW2_sb[:], fW2_d[:])

            # layer-0 table phase (from the input features)
            slab = dram.tile([NPD, D], BF16, tag="slab")
            Tfull = dram.tile([cfg.n, D], BF16, tag="Tfull")
            for nt in range(NT):
                build_table_tile(0, nt, slab)
            nc.gpsimd.collective_compute(
                "AllGather", ALU.bypass, replica_groups=rg,
                ins=[slab.opt()], outs=[Tfull.opt()],
            )

            for l in range(3):
                if l > 0:
                    slab = dram.tile([NPD, D], BF16, tag="slab")
                    Tfull = dram.tile([cfg.n, D], BF16, tag="Tfull")
                    for nt in range(NT):
                        build_table_tile(l, nt, slab)
                    nc.gpsimd.collective_compute(
                        "AllGather", ALU.bypass, replica_groups=rg,
                        ins=[slab.opt()], outs=[Tfull.opt()],
                    )

                # ---- edge phase ----
                ppool = pmiscpool.tile([P, D + 1], F32, tag="ppool")
                idxall_sb = wpool.tile([P, NT, NC // 4, 32], I16, tag="idxall")
                nc.sync.dma_start(idxall_sb[:], idxw_d.ap().rearrange("t p g w -> p t g w"))
                for t in range(NT):
                    idx_sb = idxall_sb[:, t]
                    oh_sb = ohpool.tile([P, NC, P], FP8, tag="oh")
                    nc.sync.dma_start(oh_sb[:], oh_d[t])
                    ohd_sb = ohdpool.tile([P, NE], FP8, tag="ohd")
                    nc.sync.dma_start(ohd_sb[:], ohd_d[t])

                    xlE = gpool.tile([P, NC, D], BF16, tag="xlE")
                    t_sb = tpool.tile([P, H, NE], BF16, tag="tsb")
                    plog = plogpool.tile([P, NC, 2], F32, tag="plog")

                    # ---- pass A: logits ----
                    for g in range(NG):
                        e0 = g * 512
                        nc.gpsimd.dma_gather(xlE[:, 4 * g:4 * (g + 1), :], Tfull[:],
                                             idx_sb[:, g, :], 512, 512, D,
                                             queue_num=(g % 2) * 2)
                        xlT = gtpool.tile([P, H, 512], BF16, tag="xlT")
                        nc.gpsimd.dma_gather(xlT[:], Tfull[:], idx_sb[:, g, :],
                                             512, 512, D, transpose=True,
                                             queue_num=(g % 2) * 2 + 1)
                        pm = pmpool.tile([P, H, 512], F32, tag="pm")
                        for h in range(H):
                            nc.tensor.matmul(pm[:, h, :],
                                             xr_sb[:, t, h * C:(h + 1) * C],
                                             ohd_sb[:, e0:e0 + 512],
                                             start=True, stop=False)
                            nc.tensor.matmul(pm[:, h, :], identbf[:],
                                             xlT[:, h, :], start=False, stop=True)
                            nc.scalar.activation(t_sb[:, h, e0:e0 + 512],
                                                 pm[:, h, :], AF.Prelu,
                                                 alpha=NEG_SLOPE)
                        for kl in range(4):
                            k = 4 * g + kl
                            for h in range(H):
                                nc.tensor.matmul(
                                    plog[:, k, :],
                                    t_sb[:, h, k * P:(k + 1) * P],
                                    attr_sb[:, l * 4 + h * 2:l * 4 + h * 2 + 2],
                                    start=(h == 0), stop=(h == 1))

                    # ---- exp + row form ----
                    evc = wpool.tile([P, NC * 2], BF16, tag="evc")
                    nc.scalar.activation(evc[:], plog[:], AF.Exp)
                    evT = plogpool.tile([NC * 2, P], BF16, tag="plog")
                    nc.tensor.transpose(evT[:], evc[:], identbf[:])
                    evrow = wpool.tile([NC * 2, P], BF16, tag="evrow")
                    nc.scalar.activation(evrow[:], evT[:], AF.Copy)

                    # ---- pass B: weighted aggregation ----
                    pseg = psegpool.tile([P, D + 2], F32, tag="pseg")
                    for g in range(NG):
                        evx = pevxpool.tile([P, 4, D], F32, tag="evx")
                        for kp in range(2):
                            k2 = 2 * g + kp
                            nc.tensor.matmul(evx[:, 2 * kp:2 * kp + 2, :], evrow[:],
                                             sel_sb[:, k2, :],
                                             start=True, stop=True)
                        sxl = sxlpool.tile([P, 4, D + 2], BF16, tag="sxl")
                        nc.vector.tensor_tensor(sxl[:, :, :D],
                                                xlE[:, 4 * g:4 * (g + 1), :],
                                                evx[:], ALU.mult)
                        nc.scalar.activation(sxl[:, :, D:D + 2],
                                             evc[:, 8 * g:8 * g + 8], AF.Copy)
                        for kl in range(4):
                            k = 4 * g + kl
                            nc.tensor.matmul(pseg[:], oh_sb[:, k, :],
                                             sxl[:, kl, :],
                                             start=(k == 0), stop=(k == NC - 1))

                    # ---- normalize + pool ----
                    rec = wpool.tile([P, 2], F32, tag="rec")
                    nc.vector.reciprocal(rec[:], pseg[:, D:D + 2])
                    hst = wpool.tile([P, D], BF16, tag="hst")
                    for h in range(H):
                        nc.vector.tensor_scalar(hst[:, h * C:(h + 1) * C],
                                                pseg[:, h * C:(h + 1) * C],
                                                rec[:, h:h + 1], None, ALU.mult)

                    Gt = wpool.tile([P, P], BF16, tag="Gt")
                    nc.vector.tensor_scalar(Gt[:], iotarow[:], bloc_sb[:, t:t + 1],
                                            None, ALU.is_equal)
                    nc.tensor.matmul(ppool[:, :D], Gt[:], hst[:],
                                     start=(t == 0), stop=(t == NT - 1 and l != 0))
                    if l == 0:
                        nc.tensor.matmul(ppool[:, D:D + 1], Gt[:], onescol[:],
                                         start=False, stop=(t == NT - 1))

                    # transpose h for the next layer's table build
                    if l < 2:
                        for h in range(H):
                            ptr = pmiscpool.tile([P, P], BF16, tag="ptab")
                            nc.tensor.transpose(ptr[:], hst[:, h * C:(h + 1) * C],
                                                identbf[:])
                            nc.vector.tensor_copy(hT_sb[:, h, t * P:(t + 1) * P], ptr[:])

                nc.vector.tensor_copy(pool_sb[:, l * D:(l + 1) * D], ppool[:, :D])
                if l == 0:
                    nc.vector.tensor_copy(pool_sb[:, 768:769], ppool[:, D:D + 1])

            # ------------------------------------------------------------------
            # pooling allreduce + MLP
            nc.gpsimd.indirect_dma_start(
                out=poolb_in[:],
                out_offset=IndirectOffsetOnAxis(ap=poolidx_sb[:, :1], axis=0),
                in_=pool_sb[:],
                in_offset=None,
            )
            nc.gpsimd.collective_compute(
                "AllReduce", ALU.add, replica_groups=rg,
                ins=[poolb_in.opt()], outs=[poolb_out.opt()],
            )

            poolT = [pp.tile([P, max(G, P)], F32, name=f"poolT_{kc}", tag=f"poolT_{kc}") for kc in range(7)]
            for rt in range(0, G, P):
                rows = min(P, G - rt)
                prow = wpool.tile([P, PDA], F32, tag="prow")
                nc.sync.dma_start(prow[:rows, :], poolb_out[rt:rt + rows, :])
                for cb in range(7):
                    w = min(P, PDA - cb * P)
                    ptr2 = pmiscpool.tile([P, P], F32, tag="ptab")
                    nc.tensor.transpose(ptr2[:w, :rows], prow[:rows, cb * P:cb * P + w],
                                        identf[:rows, :rows])
                    nc.vector.tensor_copy(poolT[cb][:w, rt:rt + rows], ptr2[:w, :rows])

            h1_sb = [pp.tile([P, max(G, P)], F32, name=f"h1_{mo}", tag=f"h1_{mo}") for mo in range(6)]
            for mo in range(6):
                ph1 = pmiscpool.tile([P, max(G, P)], F32, tag="ptab")
                for kc in range(7):
                    kr = min(P, PDA - kc * P)
                    nc.tensor.matmul(ph1[:, :G], fW1_sb[kc][:kr, mo * P:(mo + 1) * P],
                                     poolT[kc][:kr, :G], start=(kc == 0), stop=(kc == 6))
                nc.scalar.activation(h1_sb[mo][:, :G], ph1[:, :G], AF.Relu,
                                     bias=fb1_sb[:, mo:mo + 1])

            py = pmiscpool.tile([1, max(G, P)], F32, tag="ppool")
            for mo in range(6):
                nc.tensor.matmul(py[:, :G], fW2_sb[:, mo:mo + 1], h1_sb[mo][:, :G],
                                 start=(mo == 0), stop=(mo == 5))
            ysb = wpool.tile([1, max(G, P)], F32, tag="ysb")
            nc.vector.tensor_copy(ysb[:, :G], py[:, :G])
            nc.sync.dma_start(y_d[:], ysb[:1, :G])

    nc.compile()
    return nc


# ----------------------------------------------------------------------------
# host preprocessing
# ----------------------------------------------------------------------------

def preprocess(inputs: dict, cfg: Cfg):
    n, g, ndev = cfg.n, cfg.g, cfg.ndev
    NPD, NT = cfg.npd, cfg.nt

    x = np.asarray(inputs["x"], np.float32)
    ei = np.asarray(inputs["edge_index"]).astype(np.int64)
    batch = np.asarray(inputs["batch"]).astype(np.int64)

    src = np.concatenate([ei[0], np.arange(n)])
    dst = np.concatenate([ei[1], np.arange(n)])
    order = np.argsort(dst, kind="stable")
    src, dst = src[order], dst[order]

    # per (dev, tile) edge lists
    tile_of = dst // P              # global dst tile id (NT per device)
    counts = np.bincount(tile_of, minlength=(n // P))
    nchunk = int(np.ceil(counts.max() / P))
    nchunk = ((nchunk + 3) // 4) * 4
    cfg.nchunk = nchunk
    NE = nchunk * P

    tile_start = np.zeros(n // P + 1, np.int64)
    np.cumsum(counts, out=tile_start[1:])

    def wrap_idx(a):  # [512] int16 -> [128, 32]
        w = a.reshape(-1, 16).T.copy()          # [16, 32]
        return np.tile(w, (8, 1))               # [128, 32]

    in_maps = []
    consts = {
        "identbf": np.eye(P, dtype=BF),
        "identf": np.eye(P, dtype=np.float32),
        "ident8": np.eye(P, dtype=F8),
        "iotarow": np.tile(np.arange(P, dtype=BF)[None, :], (P, 1)),
        "ones1p": np.ones((1, P), BF),
        "onescol": np.ones((P, 1), BF),
    }
    # att as zero-padded 2-col blocks per (layer, head): logit matmuls
    # accumulate both heads into the same [128, 2] psum columns.
    att_all = np.stack([np.asarray(inputs[f"att{l+1}"], np.float32) for l in range(3)])  # [3, H, C]
    attr = np.zeros((P, 12), np.float32)
    for l in range(3):
        for h in range(H):
            attr[:, l * 4 + h * 2 + h] = att_all[l, h]
    consts["attr"] = attr.astype(BF)
    # ev-row -> [128, 2, 256] expansion selector (chunk pairs, 512-wide):
    # sel[r, k2, kl*256 + c] = (r == 2*(2*k2+kl) + c//128)
    sel = np.zeros((nchunk * 2, nchunk // 2, 2 * D), np.float32)
    for k in range(nchunk):
        for h in range(H):
            sel[2 * k + h, k // 2, (k % 2) * D + h * C:(k % 2) * D + (h + 1) * C] = 1.0
    consts["sel"] = sel.astype(BF)

    b = [np.asarray(inputs[f"b{l+1}"], np.float32) for l in range(3)]
    wmats = {}
    for l in range(3):
        wmats[f"wl{l}"] = np.asarray(inputs[f"Wl{l+1}"], np.float32).astype(BF)
        wmats[f"wr{l}"] = np.asarray(inputs[f"Wr{l+1}"], np.float32).astype(BF)
    for l in (1, 2):
        wmats[f"rl{l}"] = (b[l - 1] @ np.asarray(inputs[f"Wl{l+1}"], np.float32))[None, :].astype(BF)
        wmats[f"rr{l}"] = (b[l - 1] @ np.asarray(inputs[f"Wr{l+1}"], np.float32))[None, :].astype(BF)

    b_all = np.concatenate(b)
    fW1 = np.asarray(inputs["fW1"], np.float32)
    fW1p = np.zeros((cfg.pda, 768), np.float32)
    fW1p[:768] = fW1
    fW1p[768] = b_all @ fW1
    fb1 = np.asarray(inputs["fb1"], np.float32).reshape(6, P).T.copy()
    fW2p = np.asarray(inputs["fW2"], np.float32).reshape(6, P).T.copy()
    fb2 = float(np.asarray(inputs["fb2"]).reshape(-1)[0])

    iota_p = np.arange(P)
    for dev in range(ndev):
        lo = dev * NPD
        g_lo = int(batch[lo])
        g_hi = int(batch[lo + NPD - 1])
        assert g_hi - g_lo + 1 <= P

        idxw = np.zeros((NT, P, nchunk // 4, 32), np.int16)
        oh = np.zeros((NT, P, nchunk, P), F8)
        ohd = np.zeros((NT, P, NE), F8)
        for t in range(NT):
            gt = dev * NT + t
            s, e = tile_start[gt], tile_start[gt + 1]
            cnt = e - s
            sp = np.zeros(NE, np.int64)
            sp[:cnt] = src[s:e]
            dl = np.full(NE, -1, np.int64)
            dl[:cnt] = dst[s:e] % P
            for gi in range(nchunk // 4):
                idxw[t, :, gi, :] = wrap_idx(sp[gi * 512:(gi + 1) * 512].astype(np.int16))
            # oh[t, e_in_chunk, k, d] = (dst_local(k*128+e) == d)  (seg lhsT)
            dl2 = dl.reshape(nchunk, P)                       # [k, e]
            oh_t = (dl2[:, :, None] == iota_p[None, None, :])  # [k, e, d]
            oh[t] = oh_t.transpose(1, 0, 2).astype(F8)
            # ohd[t, d, e] = (dst_local(e) == d)  (xr-broadcast moving)
            ohd[t] = (dl[None, :] == iota_p[:, None]).astype(F8)

        bloc = (batch[lo:lo + NPD].reshape(NT, P).T - g_lo).astype(np.float32)
        poolidx = np.arange(P, dtype=np.int32) + g_lo
        poolidx[poolidx > g_hi] = g
        m = {
            "xT": x[lo:lo + NPD].T.astype(BF),
            "idxw": idxw,
            "oh": oh,
            "ohd": ohd,
            "bloc": bloc,
            "poolidx": poolidx[:, None],
            "fW1p": fW1p, "fb1p": fb1, "fW2p": fW2p,
            **consts, **wmats,
        }
        in_maps.append(m)
    return in_maps, fb2


def kernel_impl(inputs, trace=False, trace_kwargs=None):
    cfg = Cfg(n=16384, g=256, ndev=8, nchunk=0)
    in_maps, fb2 = preprocess(inputs, cfg)
    nc = build_program(cfg, fb2)
    res = run_bass_kernel_spmd(nc, in_maps, core_ids=list(range(cfg.ndev)),
                               trace=trace, **(trace_kwargs or {}))
    y = np.asarray(res.results[0]["y"], np.float32).reshape(cfg.g, 1)
    return y + fb2, res


def kernel(**inputs) -> np.ndarray:
    y, _ = kernel_impl(inputs)
    return y
